# revision 7
# baseline (speedup 1.0000x reference)
"""GCMC (NGCF-style) forward on 8 Trainium2 NeuronCores — v2.

Replaces SWDGE dma_gather (7ns/descriptor on GpSimd was the v1 wall) with
SBUF-resident tables + gpsimd ap_gather:

- Nodes split into 4 column-groups of 37632. Each layer runs 2 passes; pass t
  holds groups {2t, 2t+1} in SBUF as an f32 table [128, 2*18816]: partition
  p = (dim p%64, group-parity p//64), free = (range ri, local node).
- ap_gather (d=1, f32) pulls per-edge source values as G^T tiles: Q7 cores
  0-3 serve even-group edge streams, 4-7 odd-group, in one call.
- PE transpose turns each [128,128] G^T slice into a pair tile
  [128 edges, 64 even-dims | 64 odd-dims]; batched ACT copies convert
  PSUM->SBUF bf16.
- One-hot M matrices ([128 edges, 128 rows] bf16) are built on DVE with
  fused is_equal*val tensor_scalar ops, then bf16 matmuls accumulate
  side^T per 128-row destination block in PSUM.
- Dense 64x64 layers run per 3-block window in bf16; outputs are written
  both as transposed slabs (next layer's table, layer 0 only) and dense
  rows (BPR gathers), AllGathered in bf16.
"""
import numpy as np
import ml_dtypes

import concourse.bass as bass
import concourse.bacc as bacc
import concourse.mybir as mybir
import concourse.tile as tile
from concourse.bass_utils import run_bass_kernel_spmd
from concourse import library_config

U, I, D = 100000, 50000, 64
N = U + I
E = 3_000_000
B = 8192
NEG_SLOPE = 0.2
REG_LAMBDA = 1e-4

NCORES = 8
P = 128
NBLK = 147
RPC = P * NBLK                # 18816 rows per core
NP_ = NCORES * RPC            # 150528 padded node count
GRP = 37632                   # nodes per column-group (4 groups)
RSZ = 18816                   # nodes per range (2 ranges per group)
W = 3                         # blocks per window
NW = -(-NBLK // W)            # 49 windows
BPC = B // NCORES
BJ = BPC // P

F32 = mybir.dt.float32
BF16 = mybir.dt.bfloat16
I32 = mybir.dt.int32
I16 = mybir.dt.int16
AF = mybir.ActivationFunctionType
ALU = mybir.AluOpType
BF = ml_dtypes.bfloat16


def prep(inputs):
    rows = np.asarray(inputs["rows"], np.int64)
    cols = np.asarray(inputs["cols"], np.int64)
    vals = np.asarray(inputs["vals"], np.float32)

    core = rows // RPC
    b_loc = (rows % RPC) >> 7
    brow = (rows & 127).astype(np.float32)
    grp = cols // GRP                  # 0..3
    ri = (cols % GRP) // RSZ           # 0..1
    loc = (cols % RSZ).astype(np.int16)

    # bin = (block, grp, ri) -> chunk counts, maxed over cores
    NBIN = NBLK * 4 * 2
    key_all = (b_loc * 8 + grp * 2 + ri).astype(np.int64)
    cnts = np.zeros((NCORES, NBIN), np.int64)
    per_core = []
    for k in range(NCORES):
        m = core == k
        cnts[k] = np.bincount(key_all[m], minlength=NBIN)
        per_core.append((key_all[m], brow[m], loc[m], vals[m]))
    C = -(-np.max(cnts, axis=0) // P)       # chunks per bin

    windows = [list(range(s, min(s + W, NBLK))) for s in range(0, NBLK, W)]

    # schedule: for (t, w, ri): npair slots; chunk lists per parity
    # chunk-slot global counter s; chunk (b,g,ri) occupies slots
    # [start[bin], start[bin]+C[bin])  within its (t,w,ri) region.
    sched = []          # per (t, w): dict(ri -> (s0, npair, ev_list, od_list))
    slot_of_bin = np.zeros(NBIN, np.int64)  # first slot of bin's chunks
    NSLOT = 0
    for t in range(2):
        for wi, blocks in enumerate(windows):
            ent = {}
            for r in range(2):
                ev, od = [], []
                for b in blocks:
                    bin_e = b * 8 + (2 * t) * 2 + r
                    bin_o = b * 8 + (2 * t + 1) * 2 + r
                    for c in range(C[bin_e]):
                        ev.append((b, bin_e, c))
                    for c in range(C[bin_o]):
                        od.append((b, bin_o, c))
                npair = max(len(ev), len(od))
                s0 = NSLOT
                for j, (b, bn, c) in enumerate(ev):
                    if c == 0:
                        slot_of_bin[bn] = s0 + j
                for j, (b, bn, c) in enumerate(od):
                    if c == 0:
                        slot_of_bin[bn] = s0 + j
                NSLOT += npair
                ent[r] = (s0, npair, ev, od)
            sched.append(ent)

    # per-core streams: place each edge at (slot, lane) in its parity stream
    idx_maps, meta_maps = [], []
    for k in range(NCORES):
        key, rr_, lc_, vv_ = per_core[k]
        order = np.argsort(key, kind="stable")
        key_s, rr_s, lc_s, vv_s = key[order], rr_[order], lc_[order], vv_[order]
        gstart = np.zeros(NBIN + 1, np.int64)
        np.cumsum(np.bincount(key_s, minlength=NBIN), out=gstart[1:])
        within = np.arange(len(key_s)) - gstart[key_s]
        slot = slot_of_bin[key_s] + (within >> 7)
        lane = within & 127

        # idx: [128, NSLOT*8] int16; even stream on partitions 0..63 (4
        # cores), odd on 64..127. Within a slot, lane j of the stream is
        # packed at [16c + j%16, slot*8 + j//16].
        rr_e = np.zeros((NSLOT, P), np.float32)
        vv_e = np.zeros((NSLOT, P), np.float32)
        rr_o = np.zeros((NSLOT, P), np.float32)
        vv_o = np.zeros((NSLOT, P), np.float32)
        par = (key_s // 2) & 1
        ev_m = par == 0
        od_m = par == 1
        rr_e[slot[ev_m], lane[ev_m]] = rr_s[ev_m]
        vv_e[slot[ev_m], lane[ev_m]] = vv_s[ev_m]
        rr_o[slot[od_m], lane[od_m]] = rr_s[od_m]
        vv_o[slot[od_m], lane[od_m]] = vv_s[od_m]
        # but even/odd streams gather DIFFERENT idx on different partitions:
        idx_e = np.zeros((NSLOT, P), np.int16)
        idx_o = np.zeros((NSLOT, P), np.int16)
        idx_e[slot[ev_m], lane[ev_m]] = lc_s[ev_m]
        idx_o[slot[od_m], lane[od_m]] = lc_s[od_m]

        idx_pack = np.zeros((P, NSLOT * 8), np.int16)
        je = idx_e.reshape(NSLOT * 8, 16)   # [slotcol, 16lane]
        jo = idx_o.reshape(NSLOT * 8, 16)
        for c4 in range(4):
            idx_pack[16 * c4:16 * c4 + 16, :] = je.T
            idx_pack[64 + 16 * c4:64 + 16 * c4 + 16, :] = jo.T

        meta = np.concatenate([rr_e.T, vv_e.T, rr_o.T, vv_o.T], axis=1)
        idx_maps.append(np.ascontiguousarray(idx_pack))
        meta_maps.append(np.ascontiguousarray(meta))

    # tables / weights
    ego0 = np.concatenate([np.asarray(inputs["user_emb"], np.float32),
                           np.asarray(inputs["item_emb"], np.float32)],
                          axis=0)
    ego0_pad = np.zeros((NP_, D), np.float32)
    ego0_pad[:N] = ego0
    ego0_dense = ego0_pad.astype(BF)
    tab0 = np.ascontiguousarray(
        ego0_pad.reshape(NCORES, RPC, D).transpose(0, 2, 1).astype(BF))

    fc = np.concatenate([
        np.asarray(inputs["W_gcn0"], np.float32),
        np.asarray(inputs["W_mlp0"], np.float32),
        np.asarray(inputs["W_gcn1"], np.float32),
        np.asarray(inputs["W_mlp1"], np.float32),
        np.eye(D, dtype=np.float32),
    ], axis=1).astype(BF)
    fb = np.concatenate([
        0.8 * np.asarray(inputs["b_gcn0"], np.float32).T,
        0.2 * np.asarray(inputs["b_gcn0"], np.float32).T,
        np.asarray(inputs["b_mlp0"], np.float32).T,
        0.8 * np.asarray(inputs["b_gcn1"], np.float32).T,
        0.2 * np.asarray(inputs["b_gcn1"], np.float32).T,
        np.asarray(inputs["b_mlp1"], np.float32).T,
    ], axis=1)

    user = np.asarray(inputs["user"], np.int64)
    pos_i = np.asarray(inputs["positive"], np.int64) + U
    neg_i = np.asarray(inputs["negative"], np.int64) + U
    in_maps = []
    for k in range(NCORES):
        s0 = k * BPC
        bidx = np.concatenate([
            user[s0:s0 + BPC].reshape(P, BJ),
            pos_i[s0:s0 + BPC].reshape(P, BJ),
            neg_i[s0:s0 + BPC].reshape(P, BJ),
        ], axis=1).astype(np.int32)
        in_maps.append(dict(
            idx_all=idx_maps[k], meta=meta_maps[k],
            bidx=np.ascontiguousarray(bidx),
            ego0_dense=ego0_dense, tab0=tab0,
            fconst=np.ascontiguousarray(fc),
            fbias=np.ascontiguousarray(fb),
            identf=np.eye(P, dtype=np.float32),
            iotab=np.tile(np.arange(P, dtype=np.float32), (P, 1)).astype(BF),
        ))
    return dict(sched=sched, NSLOT=NSLOT, windows=windows), in_maps


def build(hs):
    sched, NSLOT, windows = hs["sched"], hs["NSLOT"], hs["windows"]
    maxLw = max(ent[0][1] + ent[1][1] for ent in sched)

    nc = bacc.Bacc()
    idx_all = nc.dram_tensor("idx_all", [P, NSLOT * 8], I16,
                             kind="ExternalInput")
    meta_d = nc.dram_tensor("meta", [P, 4 * NSLOT], F32,
                            kind="ExternalInput")
    iotab_d = nc.dram_tensor("iotab", [P, P], BF16, kind="ExternalInput")
    bidx_d = nc.dram_tensor("bidx", [P, 3 * BJ], I32, kind="ExternalInput")
    ego0_dense = nc.dram_tensor("ego0_dense", [NP_, D], BF16,
                                kind="ExternalInput")
    tab0 = nc.dram_tensor("tab0", [NCORES, D, RPC], BF16,
                          kind="ExternalInput")
    fconst = nc.dram_tensor("fconst", [D, 4 * D + D], BF16,
                            kind="ExternalInput")
    fbias = nc.dram_tensor("fbias", [D, 6], F32, kind="ExternalInput")
    identf_d = nc.dram_tensor("identf", [P, P], F32, kind="ExternalInput")
    out_ext = nc.dram_tensor("out", [1, 2], F32, kind="ExternalOutput")

    side0_d = nc.dram_tensor("side0", [D, RPC], F32)
    egoT1_blk = nc.dram_tensor("egoT1_blk", [D, RPC], BF16)
    egoT1_ag = nc.dram_tensor("egoT1_ag", [NCORES, D, RPC], BF16,
                              addr_space="Shared")
    dense_blk = [nc.dram_tensor(f"dense{l}_blk", [RPC, D], BF16)
                 for l in (1, 2)]
    dense_ag = [nc.dram_tensor(f"dense{l}_ag", [NP_, D], BF16,
                               addr_space="Shared") for l in (1, 2)]
    ar_in = nc.dram_tensor("ar_in", [1, 8], F32)
    ar_out = nc.dram_tensor("ar_out", [1, 8], F32, addr_space="Shared")
    RGRP = [list(range(NCORES))]

    with tile.TileContext(nc) as tc:
        nc.gpsimd.load_library(library_config.ap_gather)
        with (
            tc.tile_pool(name="const", bufs=1) as cp,
            tc.tile_pool(name="sp", bufs=2) as sp,
            tc.tile_pool(name="gp", bufs=2) as gp,
            tc.tile_pool(name="pp", bufs=2, space="PSUM") as pp,
        ):
            fc_sb = cp.tile([D, 4 * D + D], BF16)
            nc.sync.dma_start(fc_sb[:], fconst[:])
            fb_sb = cp.tile([D, 6], F32)
            nc.sync.dma_start(fb_sb[:], fbias[:])
            bidx_sb = cp.tile([P, 3 * BJ], I32)
            nc.sync.dma_start(bidx_sb[:], bidx_d[:])
            iota_sb = cp.tile([P, P], BF16)
            nc.sync.dma_start(iota_sb[:], iotab_d[:])
            identf = cp.tile([P, P], F32)
            nc.sync.dma_start(identf[:], identf_d[:])

            w_g = [fc_sb[:, 0:D], fc_sb[:, 2 * D:3 * D]]
            w_m = [fc_sb[:, D:2 * D], fc_sb[:, 3 * D:4 * D]]
            identb = fc_sb[:, 4 * D:5 * D]
            bg08 = [fb_sb[:, 0:1], fb_sb[:, 3:4]]
            bg02 = [fb_sb[:, 1:2], fb_sb[:, 4:5]]
            bm = [fb_sb[:, 2:3], fb_sb[:, 5:6]]

            tabsb = cp.tile([P, 2, RSZ, 1], F32)

            # ---- BPR gather + stats ----------------------------------
            ss, dp, dn = {}, {}, {}

            def bpr_layer(l, table):
                gbs = []
                for role in range(3):
                    g = sp.tile([P, BJ, D], BF16, tag="gb", bufs=4)
                    for j in range(BJ):
                        nc.gpsimd.indirect_dma_start(
                            out=g[:, j, :], out_offset=None, in_=table[:],
                            in_offset=bass.IndirectOffsetOnAxis(
                                ap=bidx_sb[:, role * BJ + j:role * BJ + j + 1],
                                axis=0))
                    gbs.append(g)
                for role in range(3):
                    s = cp.tile([P, BJ], F32, name=f"ss{l}_{role}")
                    for j in range(BJ):
                        sq = sp.tile([P, D], F32, tag="sq", bufs=3)
                        nc.scalar.activation(sq[:], gbs[role][:, j, :],
                                             AF.Square,
                                             accum_out=s[:, j:j + 1])
                    ss[(l, role)] = s
                for role, dst in ((1, dp), (2, dn)):
                    d_ = cp.tile([P, BJ], F32, name=f"d{l}_{role}")
                    for j in range(BJ):
                        m = sp.tile([P, D], F32, tag="dm", bufs=3)
                        nc.vector.tensor_tensor(m[:], gbs[0][:, j, :],
                                                gbs[role][:, j, :], ALU.mult)
                        nc.vector.tensor_reduce(d_[:, j:j + 1], m[:],
                                                mybir.AxisListType.X, ALU.add)
                    dst[l] = d_

            bpr_layer(0, ego0_dense)

            # ---- propagation layers ----------------------------------
            for l in range(2):
                for t in range(2):
                    for m in range(4):
                        slab = 4 * t + m
                        src = tab0[slab] if l == 0 else egoT1_ag[slab]
                        nc.gpsimd.dma_start(
                            tabsb[64 * (m // 2):64 * (m // 2) + 64,
                                  m % 2:m % 2 + 1, :, :].rearrange(
                                "p a b c -> p (a b c)"),
                            src[:, :])
                    for wi, blocks in enumerate(windows):
                        ent = sched[t * NW + wi]
                        s0w = ent[0][0]
                        Lw = ent[0][1] + ent[1][1]
                        if Lw == 0:
                            continue
                        idx_w = sp.tile([P, maxLw * 8], I16, tag="idxw")
                        nc.sync.dma_start(
                            idx_w[:, :Lw * 8],
                            idx_all[:, s0w * 8:(s0w + Lw) * 8])
                        met_w = sp.tile([P, 4, maxLw], F32, tag="metw")
                        for q in range(4):
                            nc.sync.dma_start(
                                met_w[:, q, :Lw],
                                meta_d[:, q * NSLOT + s0w:
                                       q * NSLOT + s0w + Lw])
                        GT = gp.tile([P, maxLw * P, 1], F32, tag="GT")
                        for r in range(2):
                            sr, npair, _, _ = ent[r]
                            if npair == 0:
                                continue
                            off = sr - s0w
                            nc.gpsimd.ap_gather(
                                out_ap=GT[:, off * P:(off + npair) * P, :],
                                in_ap=tabsb[:, r:r + 1, :, :].rearrange(
                                    "p a b c -> p (a b) c"),
                                idxs_ap=idx_w[:, off * 8:(off + npair) * 8],
                                channels=P, num_elems=RSZ, d=1,
                                num_idxs=npair * P)

                        # pair tiles: transpose 4 slots at a time
                        Gc = {}
                        for c0 in range(0, Lw, 4):
                            cn = min(4, Lw - c0)
                            ps = pp.tile([P, 4, P], F32, tag="tp")
                            for j in range(cn):
                                nc.tensor.transpose(
                                    ps[:, j, :],
                                    GT[:, (c0 + j) * P:(c0 + j + 1) * P, 0],
                                    identf[:])
                            gc = sp.tile([P, 4, P], BF16, tag="gc", bufs=3)
                            nc.scalar.activation(
                                gc[:, :cn, :].rearrange("p a b -> p (a b)"),
                                ps[:, :cn, :].rearrange("p a b -> p (a b)"),
                                AF.Copy)
                            Gc[c0 // 4] = gc

                        # M tiles: per-chunk fused is_equal*val (bf16)
                        Ms = {}
                        for c0 in range(0, Lw, 8):
                            cn = min(8, Lw - c0)
                            for par in range(2):
                                mt = sp.tile([P, 8, P], BF16,
                                             tag=f"M{par}", bufs=2)
                                for j in range(cn):
                                    nc.vector.tensor_scalar(
                                        mt[:, j, :], iota_sb[:],
                                        met_w[:, 2 * par,
                                              c0 + j:c0 + j + 1],
                                        met_w[:, 2 * par + 1,
                                              c0 + j:c0 + j + 1],
                                        ALU.is_equal, ALU.mult)
                                Ms[(c0 // 8, par)] = mt

                        # accumulate matmuls
                        psd = pp.tile([D, W, P], F32, tag="side")
                        first = {b: True for b in blocks}
                        nmm = {b: 0 for b in blocks}
                        for r in range(2):
                            _, _, ev, od = ent[r]
                            for lst in (ev, od):
                                for (b, bn, c) in lst:
                                    nmm[b] += 1
                        total = dict(nmm)
                        for r in range(2):
                            sr, npair, ev, od = ent[r]
                            off = sr - s0w
                            for par, lst in ((0, ev), (1, od)):
                                for j, (b, bn, c) in enumerate(lst):
                                    sl = off + j
                                    bw = blocks.index(b)
                                    nmm[b] -= 1
                                    nc.tensor.matmul(
                                        psd[:, bw, :],
                                        lhsT=Gc[sl // 4][:, sl % 4,
                                                         64 * par:
                                                         64 * par + 64],
                                        rhs=Ms[(sl // 8, par)][:, sl % 8, :],
                                        start=first[b],
                                        stop=(nmm[b] == 0))
                                    first[b] = False

                        nb = len(blocks)
                        if t == 0:
                            sp0 = sp.tile([D, W * P], F32, tag="sp0")
                            nc.scalar.activation(
                                sp0[:, :nb * P],
                                psd[:, :nb, :].rearrange("p a b -> p (a b)"),
                                AF.Copy)
                            nc.sync.dma_start(
                                side0_d[:, wi * W * P:wi * W * P + nb * P],
                                sp0[:, :nb * P])
                        else:
                            s0sb = sp.tile([D, W * P], F32, tag="s0l")
                            nc.sync.dma_start(
                                s0sb[:, :nb * P],
                                side0_d[:, wi * W * P:wi * W * P + nb * P])
                            sideT = sp.tile([D, W * P], BF16, tag="sideT")
                            nc.vector.tensor_tensor(
                                sideT[:, :nb * P],
                                psd[:, :nb, :].rearrange("p a b -> p (a b)"),
                                s0sb[:, :nb * P], ALU.add)
                            p1 = pp.tile([D, W * P], F32, tag="dns")
                            nc.tensor.matmul(p1[:, :nb * P], lhsT=w_g[l],
                                             rhs=sideT[:, :nb * P],
                                             start=True, stop=True)
                            relu8 = sp.tile([D, W * P], BF16, tag="r8")
                            nc.scalar.activation(relu8[:, :nb * P],
                                                 p1[:, :nb * P], AF.Relu,
                                                 bias=bg08[l], scale=0.8)
                            uu = sp.tile([D, W * P], BF16, tag="uu")
                            nc.vector.tensor_scalar(uu[:, :nb * P],
                                                    p1[:, :nb * P], 0.2,
                                                    bg02[l], ALU.mult,
                                                    ALU.add)
                            gcnT = sp.tile([D, W * P], BF16, tag="gcnT")
                            nc.vector.tensor_tensor(gcnT[:, :nb * P],
                                                    uu[:, :nb * P],
                                                    relu8[:, :nb * P],
                                                    ALU.add)
                            p2 = pp.tile([D, W * P], F32, tag="dns")
                            nc.tensor.matmul(p2[:, :nb * P], lhsT=w_m[l],
                                             rhs=gcnT[:, :nb * P],
                                             start=True, stop=True)
                            egoT = sp.tile([D, W * P], BF16, tag="egoT")
                            nc.scalar.activation(egoT[:, :nb * P],
                                                 p2[:, :nb * P], AF.Identity,
                                                 bias=bm[l])
                            if l == 0:
                                nc.sync.dma_start(
                                    egoT1_blk[:, wi * W * P:
                                              wi * W * P + nb * P],
                                    egoT[:, :nb * P])
                            p3 = pp.tile([P, W, D], BF16, tag="p3",
                                         bufs=1)
                            for j in range(nb):
                                nc.tensor.transpose(
                                    p3[:, j, :],
                                    egoT[:, j * P:(j + 1) * P],
                                    fc_sb[:, 4 * D:5 * D])
                            nat = sp.tile([P, W, D], BF16, tag="nat")
                            nc.scalar.activation(
                                nat[:, :nb, :].rearrange("p a b -> p (a b)"),
                                p3[:, :nb, :].rearrange("p a b -> p (a b)"),
                                AF.Copy)
                            nc.sync.dma_start(
                                dense_blk[l][wi * W * P:wi * W * P + nb * P,
                                             :].rearrange(
                                    "(a p) d -> p a d", p=P),
                                nat[:, :nb, :])

                if l == 0:
                    nc.gpsimd.collective_compute(
                        "AllGather", ALU.bypass, replica_groups=RGRP,
                        ins=[egoT1_blk[:]], outs=[egoT1_ag[:]])
                nc.gpsimd.collective_compute(
                    "AllGather", ALU.bypass, replica_groups=RGRP,
                    ins=[dense_blk[l][:]], outs=[dense_ag[l][:]])
                bpr_layer(l + 1, dense_ag[l])

            # ---- final combine --------------------------------------
            def norm_term(d_, su, so):
                tt = sp.tile([P, BJ], F32, tag="nt", bufs=6)
                nc.vector.tensor_tensor(tt[:], su[:], so[:], ALU.mult)
                t2 = sp.tile([P, BJ], F32, tag="nt", bufs=6)
                nc.scalar.activation(t2[:], tt[:], AF.Sqrt)
                t3 = sp.tile([P, BJ], F32, tag="nt", bufs=6)
                nc.vector.reciprocal(t3[:], t2[:])
                t4 = sp.tile([P, BJ], F32, tag="nt", bufs=6)
                nc.vector.tensor_tensor(t4[:], d_[:], t3[:], ALU.mult)
                return t4

            pos_s = cp.tile([P, BJ], F32)
            nc.vector.tensor_tensor(pos_s[:], dp[0][:],
                                    norm_term(dp[1], ss[(1, 0)],
                                              ss[(1, 1)])[:], ALU.add)
            nc.vector.tensor_tensor(pos_s[:], pos_s[:],
                                    norm_term(dp[2], ss[(2, 0)],
                                              ss[(2, 1)])[:], ALU.add)
            neg_s = cp.tile([P, BJ], F32)
            nc.vector.tensor_tensor(neg_s[:], dn[0][:],
                                    norm_term(dn[1], ss[(1, 0)],
                                              ss[(1, 2)])[:], ALU.add)
            nc.vector.tensor_tensor(neg_s[:], neg_s[:],
                                    norm_term(dn[2], ss[(2, 0)],
                                              ss[(2, 2)])[:], ALU.add)
            xdiff = cp.tile([P, BJ], F32)
            nc.vector.tensor_tensor(xdiff[:], neg_s[:], pos_s[:],
                                    ALU.subtract)
            ex = cp.tile([P, BJ], F32)
            nc.scalar.activation(ex[:], xdiff[:], AF.Exp)
            sp_ = cp.tile([P, BJ], F32)
            nc.scalar.activation(sp_[:], ex[:], AF.Ln, bias=1.0)

            reg_row = cp.tile([P, BJ], F32)
            nc.vector.tensor_tensor(reg_row[:], ss[(0, 0)][:],
                                    ss[(0, 1)][:], ALU.add)
            nc.vector.tensor_tensor(reg_row[:], reg_row[:], ss[(0, 2)][:],
                                    ALU.add)

            sc = cp.tile([P, 2], F32)
            srow = cp.tile([P, 1], F32)
            nc.vector.tensor_reduce(srow[:], sp_[:], mybir.AxisListType.X,
                                    ALU.add)
            nc.scalar.activation(sc[:, 0:1], srow[:], AF.Copy, scale=1.0 / B)
            rrow = cp.tile([P, 1], F32)
            nc.vector.tensor_reduce(rrow[:], reg_row[:],
                                    mybir.AxisListType.X, ALU.add)
            nc.scalar.activation(sc[:, 1:2], rrow[:], AF.Copy,
                                 scale=REG_LAMBDA * 0.5 / B)
            ones = cp.tile([P, 1], F32)
            nc.vector.memset(ones[:], 1.0)
            tot = pp.tile([1, 2], F32, tag="tot", bufs=1)
            nc.tensor.matmul(tot[:], lhsT=ones[:], rhs=sc[:], start=True,
                             stop=True)
            ar_sb = cp.tile([1, 8], F32)
            nc.vector.memset(ar_sb[:], 0.0)
            nc.scalar.copy(ar_sb[:, 0:2], tot[:])
            nc.sync.dma_start(ar_in[:], ar_sb[:])
            nc.gpsimd.collective_compute(
                "AllReduce", ALU.add, replica_groups=RGRP,
                ins=[ar_in[:]], outs=[ar_out[:]])
            nc.sync.dma_start(out_ext[:], ar_out[:1, 0:2])
    nc.compile()
    return nc


def run(inputs, trace=False, trace_cores=None):
    inputs = {k: np.asarray(v) for k, v in inputs.items()}
    hs, in_maps = prep(inputs)
    nc = build(hs)
    kw = {}
    if trace:
        kw = dict(trace=True, trace_cores=trace_cores or [0])
    res = run_bass_kernel_spmd(nc, in_maps, list(range(NCORES)), **kw)
    out = res.results[0]["out"].reshape(2).astype(np.float32)
    return out, res


def kernel(**inputs):
    out, _ = run(inputs)
    return out
